# revision 2
# baseline (speedup 1.0000x reference)
"""Trainium2 Bass kernel for ContextualLanguageRefinement.

Math (per batch b):
  Q = h @ W_Q / sqrt(DS); K = h @ W_K
  scores[t,s] = Q[t].K[s], banded |t-s|<=3, softmax over s
  out = softmax((attn @ h @ W_proj) / tau)  using  attn @ (h @ W_proj)

Sharding: data-parallel over batch B=8 across the 8 NeuronCores; the small
weights are replicated (concatenated + pre-scaled + bf16-cast on host).

Per-core device pipeline:
  1. h [2048,1024] f32 is DMA-loaded in a 32x32-block-swizzled layout
     (contiguous 128B runs), cast to bf16 on gpsimd, and block-transposed on
     the vector engine -> hT [d, t] bf16.
  2. One fused projection YT = Wcat^T @ hT gives Q^T, K^T (padded), hp^T.
  3. Per t-block of 120: banded scores via PE, mask+exp, then the
     attention-weighted combine and softmax denominator come from a single
     PE matmul against hp windows augmented with a ones column.
"""

import numpy as np
import ml_dtypes

import concourse.bass as bass
import concourse.bacc as bacc
import concourse.tile as tile
from concourse import mybir
from concourse.bass_utils import run_bass_kernel_spmd

F32 = mybir.dt.float32
BF16 = mybir.dt.bfloat16

B, T, D = 8, 2048, 1024
DS, KL = 256, 32
WIN = 3
SCALE = float(np.sqrt(DS))
MW = 2 * DS + KL          # 544 concatenated output cols
TB = 120                  # tokens per block
NB = (T + TB - 1) // TB   # 18 blocks (17 full + tail of 8)
PAD = 8                   # zero padding cols on each side of K^T / hp^T
NEG = -1e9

N_CORES = 8


def build_nc():
    nc = bacc.Bacc("TRN2", target_bir_lowering=False, debug=False)

    h_d = nc.dram_tensor("h", [T, D], F32, kind="ExternalInput")
    w_d = nc.dram_tensor("wcat", [D, MW], BF16, kind="ExternalInput")
    o_d = nc.dram_tensor("out", [T, KL], F32, kind="ExternalOutput")

    with tile.TileContext(nc) as tc:
        with (
            tc.tile_pool(name="persist", bufs=1) as pp,
            tc.tile_pool(name="y32", bufs=2) as yp,
            tc.tile_pool(name="yb", bufs=2) as ybp,
            tc.tile_pool(name="blk", bufs=3) as bp,
            tc.tile_pool(name="ppsum", bufs=2, space="PSUM") as ppsum,
            tc.tile_pool(name="tpsum", bufs=2, space="PSUM") as tpsum,
            tc.tile_pool(name="spsum", bufs=2, space="PSUM") as spsum,
            tc.tile_pool(name="lpsum", bufs=2, space="PSUM") as lpsum,
        ):
            # ---------------- persistent tiles ----------------
            wc = pp.tile([128, 8, MW], BF16, tag="wc")
            hbt = pp.tile([128, 8, T], BF16, tag="hbt")
            qt = pp.tile([128, 2, T], BF16, tag="qt")
            ktp = pp.tile([128, 2, T + 2 * PAD], BF16, tag="ktp")
            hpt = pp.tile([32, T + 2 * PAD], BF16, tag="hpt")
            m_mid = pp.tile([128, TB], F32, tag="m_mid")
            m_first = pp.tile([128, TB], F32, tag="m_first")
            m_last = pp.tile([20, 8], F32, tag="m_last")
            ident = pp.tile([32, 32], BF16, tag="ident")

            # masks: band is 1 <= p - f <= 7 (s = t0-4+p, t = t0+f)
            for msk, wdt in ((m_mid, TB), (m_first, TB)):
                nc.gpsimd.memset(msk[:], 0.0)
                nc.gpsimd.affine_select(
                    out=msk[:], in_=msk[:], compare_op=mybir.AluOpType.is_ge,
                    fill=NEG, base=-1, channel_multiplier=1, pattern=[[-1, wdt]])
                nc.gpsimd.affine_select(
                    out=msk[:], in_=msk[:], compare_op=mybir.AluOpType.is_ge,
                    fill=NEG, base=7, channel_multiplier=-1, pattern=[[1, wdt]])
            # first block: s = p - 4 < 0 invalid -> kill rows p < 4
            nc.gpsimd.affine_select(
                out=m_first[:], in_=m_first[:], compare_op=mybir.AluOpType.is_ge,
                fill=NEG, base=-4, channel_multiplier=1, pattern=[[0, TB]])
            # last block (t0=2040, M=20): band plus s = 2036+p < 2048 -> p <= 11
            nc.gpsimd.memset(m_last[:], 0.0)
            nc.gpsimd.affine_select(
                out=m_last[:], in_=m_last[:], compare_op=mybir.AluOpType.is_ge,
                fill=NEG, base=-1, channel_multiplier=1, pattern=[[-1, 8]])
            nc.gpsimd.affine_select(
                out=m_last[:], in_=m_last[:], compare_op=mybir.AluOpType.is_ge,
                fill=NEG, base=7, channel_multiplier=-1, pattern=[[1, 8]])
            nc.gpsimd.affine_select(
                out=m_last[:], in_=m_last[:], compare_op=mybir.AluOpType.is_ge,
                fill=NEG, base=11, channel_multiplier=-1, pattern=[[0, 8]])

            # identity (bf16) for PE transposes of hp^T windows
            nc.gpsimd.memset(ident[:], 0.0)
            nc.gpsimd.affine_select(
                out=ident[:], in_=ident[:], compare_op=mybir.AluOpType.not_equal,
                fill=1.0, base=0, channel_multiplier=1, pattern=[[-1, 32]])

            # zero padding columns of ktp / hpt
            nc.vector.memset(ktp[:, :, 0:PAD], 0.0)
            nc.vector.memset(ktp[:, :, T + PAD:T + 2 * PAD], 0.0)
            nc.vector.memset(hpt[:, 0:PAD], 0.0)
            nc.vector.memset(hpt[:, T + PAD:T + 2 * PAD], 0.0)

            # weights: wc[p, c, m] = wcat[128c + p, m]
            nc.sync.dma_start(out=wc[:], in_=bass.AP(
                tensor=w_d[:].tensor, offset=0,
                ap=[[MW, 128], [128 * MW, 8], [1, MW]]))

            # ---------------- 1. swizzled load + cast + transpose ----------------
            # y[32q+ti, k, di] = h[32k+ti, 128c+32q+di]; block-transpose -> hT
            for c in range(8):
                y32 = yp.tile([128, T // 32, 32], F32, tag="y32")
                for q in range(4):
                    in_ap = bass.AP(
                        tensor=h_d[:].tensor, offset=128 * c + 32 * q,
                        ap=[[D, 32], [32 * D, T // 32], [1, 32]])
                    nc.sync.dma_start(out=y32[32 * q:32 * (q + 1)], in_=in_ap)
                yb = ybp.tile([128, T], BF16, tag="yb")
                nc.gpsimd.tensor_copy(out=yb[:], in_=y32[:].rearrange("p k d -> p (k d)"))
                nc.vector.transpose(out=hbt[:, c, :], in_=yb[:])

            # ---------------- 2. fused projection ----------------
            # YT[m, t] = sum_c wc[:, c, m]^T @ hbt[:, c, t]
            for m in range(5):
                moff, mp = (m * 128, 128) if m < 4 else (512, 32)
                for j in range(T // 512):
                    ps = ppsum.tile([mp, 512], F32, tag="pps")
                    for c in range(8):
                        nc.tensor.matmul(
                            ps[:], wc[:, c, moff:moff + mp],
                            hbt[:, c, 512 * j:512 * (j + 1)],
                            start=(c == 0), stop=(c == 7))
                    if m < 2:
                        dst = qt[:, m, 512 * j:512 * (j + 1)]
                    elif m < 4:
                        dst = ktp[:, m - 2, PAD + 512 * j:PAD + 512 * (j + 1)]
                    else:
                        dst = hpt[:, PAD + 512 * j:PAD + 512 * (j + 1)]
                    if (m + j) % 2 == 0:
                        nc.vector.tensor_copy(out=dst, in_=ps[:])
                    else:
                        nc.scalar.copy(out=dst, in_=ps[:])

            # ---------------- 3. per-block attention ----------------
            for i in range(NB):
                t0 = TB * i
                w = TB if i < NB - 1 else T - TB * (NB - 1)   # 120 or 8
                M = 128 if i < NB - 1 else 20                 # s-window width
                wst = t0 + 4  # window start col in padded coords (s = t0 - 4)

                # hp window [M, 33]: transpose hpt cols, append ones column
                pst = tpsum.tile([M, 32], BF16, tag="pst")
                nc.tensor.transpose(pst[:], hpt[:, wst:wst + M], ident[:])
                hpw = bp.tile([M, 33], BF16, tag="hpw")
                nc.scalar.copy(out=hpw[:, 0:32], in_=pst[:])
                nc.vector.memset(hpw[:, 32:33], 1.0)

                # banded scores S^T[s, t] (2 ds-chunk accumulation)
                sps = spsum.tile([M, w], F32, tag="sps")
                for dsc in range(2):
                    nc.tensor.matmul(
                        sps[:], ktp[:, dsc, wst:wst + M],
                        qt[:, dsc, t0:t0 + w],
                        start=(dsc == 0), stop=(dsc == 1))

                # mask + exp -> EST bf16
                msk = m_first if i == 0 else (m_last if i == NB - 1 else m_mid)
                sm = bp.tile([M, w], F32, tag="sm")
                nc.vector.tensor_add(out=sm[:], in0=sps[:], in1=msk[0:M, 0:w])
                est = bp.tile([M, w], BF16, tag="est")
                nc.scalar.activation(out=est[:], in_=sm[:],
                                     func=mybir.ActivationFunctionType.Exp)

                # combine: [w, 33] = EST^T @ [hp | 1]
                lps = lpsum.tile([w, 33], F32, tag="lps")
                nc.tensor.matmul(lps[:], est[:], hpw[:], start=True, stop=True)

                # normalize by band sum (col 32), then softmax over 32 logits
                r = bp.tile([w, 1], F32, tag="r")
                nc.vector.reciprocal(out=r[:], in_=lps[:, 32:33])
                pe = bp.tile([w, KL], F32, tag="pe")
                se = bp.tile([w, 1], F32, tag="se")
                nc.scalar.activation(out=pe[:], in_=lps[:, 0:KL],
                                     func=mybir.ActivationFunctionType.Exp,
                                     scale=r[:], accum_out=se[:])
                rs = bp.tile([w, 1], F32, tag="rs")
                nc.vector.reciprocal(out=rs[:], in_=se[:])
                ot = bp.tile([w, KL], F32, tag="ot")
                nc.vector.tensor_scalar_mul(out=ot[:], in0=pe[:], scalar1=rs[:])
                nc.sync.dma_start(out=o_d[t0:t0 + w, :], in_=ot[:])

    nc.compile()
    return nc


_NC_CACHE = {}


def _get_nc():
    if "nc" not in _NC_CACHE:
        _NC_CACHE["nc"] = build_nc()
    return _NC_CACHE["nc"]


def kernel(h_base, tau, W_Q, W_K, W_proj):
    h_base = np.asarray(h_base, dtype=np.float32)
    tau_f = float(np.asarray(tau))
    wcat = np.concatenate(
        [np.asarray(W_Q, np.float32) / SCALE,
         np.asarray(W_K, np.float32),
         np.asarray(W_proj, np.float32) / tau_f], axis=1
    ).astype(ml_dtypes.bfloat16)

    nc = _get_nc()
    in_maps = [
        {"h": np.ascontiguousarray(h_base[b]), "wcat": wcat}
        for b in range(B)
    ]
    res = run_bass_kernel_spmd(nc, in_maps, list(range(N_CORES)))
    return np.stack([np.asarray(res.results[b]["out"], dtype=np.float32)
                     for b in range(B)])
